# revision 1
# baseline (speedup 1.0000x reference)
"""SATD loss kernel for Trainium2: sum |H @ (original - pred)|.

Full inputs: original, pred [2, 8192, 64, 64] f32. H is the 64x64
Sylvester Hadamard matrix applied along axis -2 of each 64x64 block.

Strategy (8-way data parallel over the 16384 blocks, 2048 per core):
  - Host: shard blocks across cores, cast to bf16 (H has +-1 entries;
    the transform accumulates in fp32 PSUM, and the bf16 input rounding
    contributes ~1e-6 relative error on the final scalar), and repack
    each core's data into [T, 128, 2*COLS] tiles whose partition axis
    holds the j-rows of 128 blocks (two 64-block halves m=0/1 on
    partitions 0-63 / 64-127) and whose free axis is (g, k) for
    original then pred.
  - Device, per tile: one contiguous 4 MiB DMA; per 512-column slice,
    matmul with lhsT = kron(I2, H) on the original half, then
    accumulate matmul with -kron(I2, H) on the pred half into the same
    PSUM bank -> PSUM = H @ (A - B) for 16 blocks at 128 partitions.
  - Fused abs+sum (tensor_reduce apply_absolute_value on VectorE) per
    PSUM bank into an SBUF accumulator; final reduce -> [128, 1]/core.
  - Host sums the 8x128 partials (f64) and casts to f32.
"""

from contextlib import ExitStack

import ml_dtypes
import numpy as np

import concourse.bass as bass
import concourse.tile as tile
from concourse import bacc, mybir
from concourse.bass_utils import run_bass_kernel_spmd

N_CORES = 8
N = 64                       # Hadamard block size
BLOCKS_TOTAL = 2 * 8192      # 16384 blocks of [64, 64]
BLOCKS_PER_CORE = BLOCKS_TOTAL // N_CORES   # 2048
G = 128                      # blocks per partition-half per tile
COLS = G * N                 # 4096 bf16 = 8 KiB per partition per input
TILES = BLOCKS_PER_CORE // (2 * G)          # 16 iterations
MM_N = 512                   # matmul moving free dim (one PSUM bank)
SUB = COLS // MM_N           # psum tiles per SBUF tile (8)

F32 = mybir.dt.float32
# Input quantization: "bf16" (rel err ~1e-6) or "e4m3" (rel err ~4e-4,
# half the DMA traffic). PSUM accumulation is fp32 either way.
import os
QUANT = os.environ.get("SATD_QUANT", "e4m3")
if QUANT == "e4m3":
    IN_DT = mybir.dt.float8e4
    IN_NP = ml_dtypes.float8_e4m3
else:
    IN_DT = mybir.dt.bfloat16
    IN_NP = ml_dtypes.bfloat16


def _hadamard(n: int) -> np.ndarray:
    H = np.array([[1.0]], dtype=np.float32)
    while H.shape[0] < n:
        H = np.block([[H, H], [H, -H]])
    return H.astype(np.float32)


def _weights() -> np.ndarray:
    # lhsT for out = Hd @ rhs is Hd.T; kron(I2, H) is symmetric.
    Hd = np.kron(np.eye(2, dtype=np.float32), _hadamard(N))
    return np.concatenate([Hd, -Hd], axis=1).astype(
        IN_NP)  # [128, 256], entries +-1 exact in bf16/e4m3


def _build_program() -> bacc.Bacc:
    nc = bacc.Bacc("TRN2", target_bir_lowering=False, debug=False,
                   num_devices=N_CORES)
    x = nc.dram_tensor("x", [TILES, 128, 2 * COLS], IN_DT,
                       kind="ExternalInput").ap()
    w = nc.dram_tensor("w", [128, 256], IN_DT, kind="ExternalInput").ap()
    out = nc.dram_tensor("out", [128, 2], F32, kind="ExternalOutput").ap()

    with tile.TileContext(nc) as tc, ExitStack() as ctx:
        wpool = ctx.enter_context(tc.tile_pool(name="w", bufs=1))
        xpool = ctx.enter_context(tc.tile_pool(name="x", bufs=3))
        psum = ctx.enter_context(tc.tile_pool(name="psum", bufs=4,
                                              space="PSUM"))
        accpool = ctx.enter_context(tc.tile_pool(name="acc", bufs=1))
        scratch = ctx.enter_context(tc.tile_pool(name="scr", bufs=2))

        wt = wpool.tile([128, 256], IN_DT)
        nc.sync.dma_start(wt[:], w[:])
        w_pos = wt[:, 0:128]
        w_neg = wt[:, 128:256]

        # Separate accumulators per reduce engine so VectorE and ScalarE
        # never touch the same tile (no cross-engine serialization).
        npairs = TILES * SUB // 2
        accv = accpool.tile([128, 5 * (npairs // 8)], F32, tag="accv")
        acca = accpool.tile([128, 3 * (npairs // 8)], F32, tag="acca")

        w3 = wt[:].rearrange("p (h m) -> p h m", h=2)

        for t in range(TILES):
            xt = xpool.tile([128, 2 * COLS], IN_DT)
            # Host layout interleaves the original/pred halves per
            # 512-column group: xt cols = (s, h, c). Any contiguous
            # column range is then self-contained, so the first and
            # last tiles stream in chunks (faster pipeline fill/drain)
            # while middle tiles use one large DMA.
            n_chunks = 4 if t in (0, TILES - 1) else 1
            step = SUB // n_chunks
            for c0 in range(0, SUB, step):
                lo, hi = c0 * 2 * MM_N, (c0 + step) * 2 * MM_N
                nc.sync.dma_start(xt[:, lo:hi], x[t, :, lo:hi])
            # DoubleRow contracts over (p, h) in one pass: a single
            # matmul computes Hd@A - Hd@B per 512-column group. Pairs of
            # groups share a 2-bank PSUM tile and one abs+sum; VectorE
            # (lower per-op cost) takes 5 of every 8 pairs, ScalarE 3.
            for pr in range(SUB // 2):
                pt = psum.tile([128, 2 * MM_N], F32)
                for q in range(2):
                    s = pr * 2 + q
                    x3 = xt[:, s * 2 * MM_N:(s + 1) * 2 * MM_N].rearrange(
                        "p (h c) -> p h c", h=2)
                    nc.tensor.matmul(pt[:, q * MM_N:(q + 1) * MM_N], w3, x3,
                                     start=True, stop=True,
                                     perf_mode=mybir.MatmulPerfMode.DoubleRow)
                k = t * (SUB // 2) + pr
                if pr % 8 < 5:
                    col = 5 * (k // 8) + k % 8
                    nc.vector.tensor_reduce(
                        accv[:, col:col + 1], pt[:],
                        axis=mybir.AxisListType.X, op=mybir.AluOpType.add,
                        apply_absolute_value=True)
                else:
                    col = 3 * (k // 8) + k % 8 - 5
                    st = scratch.tile([128, 2 * MM_N], F32)
                    nc.scalar.activation(
                        st[:], pt[:], mybir.ActivationFunctionType.Abs,
                        accum_out=acca[:, col:col + 1])

        res = accpool.tile([128, 2], F32, tag="res")
        nc.vector.tensor_reduce(res[:, 0:1], accv[:],
                                axis=mybir.AxisListType.X,
                                op=mybir.AluOpType.add)
        nc.vector.tensor_reduce(res[:, 1:2], acca[:],
                                axis=mybir.AxisListType.X,
                                op=mybir.AluOpType.add)
        nc.sync.dma_start(out[:], res[:])

    nc.compile()
    return nc


def _repack(shard: np.ndarray) -> np.ndarray:
    """[BLOCKS_PER_CORE, 64, 64] -> [TILES, 128, SUB, COLS // SUB] with
    partition axis (m, j) and free axis (g, k) split into SUB groups of
    512 columns (8 g-blocks each)."""
    v = shard.reshape(TILES, 2, G, N, N)          # t, m, g, j, k
    v = v.transpose(0, 1, 3, 2, 4)                # t, m, j, g, k
    return v.reshape(TILES, 128, SUB, MM_N)


_NC = None


def _get_program() -> bacc.Bacc:
    global _NC
    if _NC is None:
        _NC = _build_program()
    return _NC


def _run(original: np.ndarray, pred: np.ndarray, **spmd_kwargs):
    a_full = np.asarray(original, dtype=np.float32).reshape(
        BLOCKS_TOTAL, N, N).astype(IN_NP)
    b_full = np.asarray(pred, dtype=np.float32).reshape(
        BLOCKS_TOTAL, N, N).astype(IN_NP)
    wnp = _weights()
    in_maps = []
    for i in range(N_CORES):
        sl = slice(i * BLOCKS_PER_CORE, (i + 1) * BLOCKS_PER_CORE)
        xi = np.empty((TILES, 128, SUB, 2, MM_N), dtype=IN_NP)
        xi[:, :, :, 0, :] = _repack(a_full[sl])
        xi[:, :, :, 1, :] = _repack(b_full[sl])
        in_maps.append({"x": xi.reshape(TILES, 128, 2 * COLS), "w": wnp})
    nc = _get_program()
    r = run_bass_kernel_spmd(nc, in_maps, list(range(N_CORES)),
                             **spmd_kwargs)
    total = 0.0
    for i in range(N_CORES):
        total += r.results[i]["out"].astype(np.float64).sum()
    return np.float32(total), r


def kernel(original: np.ndarray, pred: np.ndarray) -> np.ndarray:
    val, _ = _run(original, pred)
    return np.array(val, dtype=np.float32)



# revision 2
# speedup vs baseline: 1.0178x; 1.0178x over previous
"""SATD loss kernel for Trainium2: sum |H @ (original - pred)|.

Full inputs: original, pred [2, 8192, 64, 64] f32. H is the 64x64
Sylvester Hadamard matrix applied along axis -2 of each 64x64 block.

Strategy (8-way data parallel over the 16384 blocks, 2048 per core):
  - Host: diff = original - pred in f32, quantized to e4m3 (H is
    linear, so sum|H@orig - H@pred| == sum|H@diff|; quantizing the
    difference once is both cheaper and more accurate than quantizing
    the operands separately). Repack each core's 2048 blocks into
    [T, 128, COLS] tiles: partition axis holds (m, j) = 2 stacked
    blocks x 64 rows, free axis is (g, k) groups of 512 columns.
  - Device, per 512-column group: one fp8 DoubleRow matmul with
    lhsT = [Hd/2 | Hd/2] ([128, 2, 128], Hd = kron(I2, H)) and the
    rhs group broadcast on the h axis (zero-stride), which computes
    Hd @ D at 0.5 PE cycles per output column. Four groups accumulate
    nothing -- each lands in its own quarter of a 4-bank PSUM quad.
  - Fused abs+sum per quad: VectorE tensor_reduce(abs) directly from
    PSUM, or ScalarE activation(Abs, accum_out) (writes a discarded
    bf16 scratch); quads are split between the two engines in a
    measured ratio. Final reduce -> [128, 2] per core.
  - Host sums the 8x256 partials (f64) and casts to f32.
"""

import os
from contextlib import ExitStack

import ml_dtypes
import numpy as np

import concourse.bass as bass
import concourse.tile as tile
from concourse import bacc, mybir
from concourse.bass_utils import run_bass_kernel_spmd

N_CORES = 8
N = 64                       # Hadamard block size
BLOCKS_TOTAL = 2 * 8192      # 16384 blocks of [64, 64]
BLOCKS_PER_CORE = BLOCKS_TOTAL // N_CORES   # 2048
TILES = 16                   # DMA tiles per core
G = BLOCKS_PER_CORE // (2 * TILES)          # 64 column-groups of 64 per tile
COLS = G * N                 # 4096 fp8 = 4 KiB per partition per tile
MM_N = 512                   # matmul moving free dim (one PSUM bank)
QUAD = 4 * MM_N              # reduce granularity: 4 banks = 2048 f32
QPT = COLS // QUAD           # PSUM quads per tile (2)

F32 = mybir.dt.float32
IN_DT = mybir.dt.float8e4
IN_NP = ml_dtypes.float8_e4m3

MM_MODE = os.environ.get("SATD_MM", "dr0")       # dr0 | plain
DVE_QUADS = int(os.environ.get("SATD_DVE", "15"))  # of every 32 quads


def _hadamard(n: int) -> np.ndarray:
    H = np.array([[1.0]], dtype=np.float32)
    while H.shape[0] < n:
        H = np.block([[H, H], [H, -H]])
    return H.astype(np.float32)


def _weights() -> np.ndarray:
    Hd = np.kron(np.eye(2, dtype=np.float32), _hadamard(N))
    if MM_MODE == "dr0":
        # DoubleRow lhsT [128, 2*128]: both halves Hd/2; the rhs h axis
        # is a zero-stride broadcast, so out = (Hd/2 + Hd/2) @ D.
        return np.concatenate([Hd / 2, Hd / 2], axis=1).astype(IN_NP)
    return Hd.astype(IN_NP)  # [128, 128]


def _build_program() -> bacc.Bacc:
    nc = bacc.Bacc("TRN2", target_bir_lowering=False, debug=False,
                   num_devices=N_CORES)
    x = nc.dram_tensor("x", [TILES, 128, COLS], IN_DT,
                       kind="ExternalInput").ap()
    wshape = [128, 256] if MM_MODE == "dr0" else [128, 128]
    w = nc.dram_tensor("w", wshape, IN_DT, kind="ExternalInput").ap()
    out = nc.dram_tensor("out", [128, 2], F32, kind="ExternalOutput").ap()

    nquads = TILES * QPT                     # 32
    # Stable interleave of the DVE/Act split across the quad stream.
    dve_q = {i for i in range(nquads)
             if (i + 1) * DVE_QUADS // nquads > i * DVE_QUADS // nquads}
    n_dve = len(dve_q)
    n_act = nquads - n_dve

    with tile.TileContext(nc) as tc, ExitStack() as ctx:
        wpool = ctx.enter_context(tc.tile_pool(name="w", bufs=1))
        xpool = ctx.enter_context(tc.tile_pool(name="x", bufs=4))
        psum = ctx.enter_context(tc.tile_pool(name="psum", bufs=2,
                                              space="PSUM"))
        accpool = ctx.enter_context(tc.tile_pool(name="acc", bufs=1))
        scratch = ctx.enter_context(tc.tile_pool(name="scr", bufs=2))

        wt = wpool.tile(wshape, IN_DT)
        nc.sync.dma_start(wt[:], w[:])
        if MM_MODE == "dr0":
            w3 = wt[:].rearrange("p (h m) -> p h m", h=2)

        # Separate accumulators per reduce engine so VectorE and ScalarE
        # never touch the same tile (no cross-engine serialization).
        accv = accpool.tile([128, n_dve], F32, tag="accv")
        acca = accpool.tile([128, n_act], F32, tag="acca")

        iv = ia = 0
        for t in range(TILES):
            xt = xpool.tile([128, COLS], IN_DT)
            # First tile streams in chunks for faster pipeline fill.
            n_chunks = 4 if t == 0 else 1
            step = COLS // n_chunks
            for c0 in range(0, COLS, step):
                nc.sync.dma_start(xt[:, c0:c0 + step], x[t, :, c0:c0 + step])
            for qq in range(QPT):
                pt = psum.tile([128, QUAD], F32)
                for s in range(QUAD // MM_N):
                    lo = qq * QUAD + s * MM_N
                    xs = xt[:, lo:lo + MM_N]
                    po = pt[:, s * MM_N:(s + 1) * MM_N]
                    if MM_MODE == "dr0":
                        x3 = xs.unsqueeze(1).broadcast_to([128, 2, MM_N])
                        nc.tensor.matmul(
                            po, w3, x3, start=True, stop=True,
                            perf_mode=mybir.MatmulPerfMode.DoubleRow)
                    else:
                        nc.tensor.matmul(po, wt[:], xs, start=True,
                                         stop=True)
                k = t * QPT + qq
                if k in dve_q:
                    nc.vector.tensor_reduce(
                        accv[:, iv:iv + 1], pt[:],
                        axis=mybir.AxisListType.X, op=mybir.AluOpType.add,
                        apply_absolute_value=True)
                    iv += 1
                else:
                    st = scratch.tile([128, QUAD], mybir.dt.bfloat16)
                    nc.scalar.activation(
                        st[:], pt[:], mybir.ActivationFunctionType.Abs,
                        accum_out=acca[:, ia:ia + 1])
                    ia += 1

        res = accpool.tile([128, 2], F32, tag="res")
        nc.vector.tensor_reduce(res[:, 0:1], accv[:],
                                axis=mybir.AxisListType.X,
                                op=mybir.AluOpType.add)
        nc.vector.tensor_reduce(res[:, 1:2], acca[:],
                                axis=mybir.AxisListType.X,
                                op=mybir.AluOpType.add)
        nc.sync.dma_start(out[:], res[:])

    nc.compile()
    return nc


def _repack(shard: np.ndarray) -> np.ndarray:
    """[BLOCKS_PER_CORE, 64, 64] f32 -> [TILES, 128, COLS] fp8 with
    partition axis (m, j) and free axis (g, k)."""
    v = shard.reshape(TILES, 2, G, N, N)          # t, m, g, j, k
    v = v.transpose(0, 1, 3, 2, 4)                # t, m, j, g, k
    return np.ascontiguousarray(v).reshape(TILES, 128, COLS).astype(IN_NP)


_NC = None


def _get_program() -> bacc.Bacc:
    global _NC
    if _NC is None:
        _NC = _build_program()
    return _NC


def _run(original: np.ndarray, pred: np.ndarray, **spmd_kwargs):
    diff = np.asarray(original, dtype=np.float32).reshape(
        BLOCKS_TOTAL, N, N) - np.asarray(pred, dtype=np.float32).reshape(
        BLOCKS_TOTAL, N, N)
    wnp = _weights()
    in_maps = []
    for i in range(N_CORES):
        sl = slice(i * BLOCKS_PER_CORE, (i + 1) * BLOCKS_PER_CORE)
        in_maps.append({"x": _repack(diff[sl]), "w": wnp})
    nc = _get_program()
    r = run_bass_kernel_spmd(nc, in_maps, list(range(N_CORES)),
                             **spmd_kwargs)
    total = 0.0
    for i in range(N_CORES):
        total += r.results[i]["out"].astype(np.float64).sum()
    return np.float32(total), r


def kernel(original: np.ndarray, pred: np.ndarray) -> np.ndarray:
    val, _ = _run(original, pred)
    return np.array(val, dtype=np.float32)


# revision 3
# speedup vs baseline: 1.2317x; 1.2102x over previous
"""SATD loss kernel for Trainium2: sum |H @ (original - pred)|.

Full inputs: original, pred [2, 8192, 64, 64] f32. H is the 64x64
Sylvester Hadamard matrix applied along axis -2 of each 64x64 block.

Strategy (8-way data parallel over the 16384 blocks, 2048 per core):
  - Host: diff = original - pred in f32, quantized to e4m3 (H is
    linear, so sum|H@orig - H@pred| == sum|H@diff|; quantizing the
    difference once is both cheaper and more accurate than quantizing
    the operands separately). Repack each core's 2048 blocks into
    [T, 128, COLS] tiles: partition axis holds (m, j) = 2 stacked
    blocks x 64 rows, free axis is (g, k) groups of 512 columns.
  - Device, per 512-column group: one fp8 DoubleRow matmul with
    lhsT = [Hd/2 | Hd/2] ([128, 2, 128], Hd = kron(I2, H)) and the
    rhs group broadcast on the h axis (zero-stride), which computes
    Hd @ D at 0.5 PE cycles per output column. Four groups accumulate
    nothing -- each lands in its own quarter of a 4-bank PSUM quad.
  - Fused abs+sum per quad: VectorE tensor_reduce(abs) directly from
    PSUM, or ScalarE activation(Abs, accum_out) (writes a discarded
    bf16 scratch); quads are split between the two engines in a
    measured ratio. Final reduce -> [128, 2] per core.
  - Host sums the 8x256 partials (f64) and casts to f32.
"""

import os
from contextlib import ExitStack

import ml_dtypes
import numpy as np

import concourse.bass as bass
import concourse.tile as tile
from concourse import bacc, mybir
from concourse.bass_utils import run_bass_kernel_spmd

N_CORES = 8
N = 64                       # Hadamard block size
BLOCKS_TOTAL = 2 * 8192      # 16384 blocks of [64, 64]
BLOCKS_PER_CORE = BLOCKS_TOTAL // N_CORES   # 2048
TILES = 16                   # DMA tiles per core
G = BLOCKS_PER_CORE // (2 * TILES)          # 64 column-groups of 64 per tile
COLS = G * N                 # 4096 fp8 = 4 KiB per partition per tile
MM_N = 512                   # matmul moving free dim (one PSUM bank)
QUAD = 2 * MM_N              # reduce granularity: 2 banks = 1024 f32
QPT = COLS // QUAD           # PSUM pairs per tile (4)

F32 = mybir.dt.float32
IN_DT = mybir.dt.float8e4
IN_NP = ml_dtypes.float8_e4m3

MM_MODE = os.environ.get("SATD_MM", "dr0")       # dr0 | plain
DVE_QUADS = int(os.environ.get("SATD_DVE", "34"))  # of every 64 pairs


def _hadamard(n: int) -> np.ndarray:
    H = np.array([[1.0]], dtype=np.float32)
    while H.shape[0] < n:
        H = np.block([[H, H], [H, -H]])
    return H.astype(np.float32)


def _weights() -> np.ndarray:
    Hd = np.kron(np.eye(2, dtype=np.float32), _hadamard(N))
    if MM_MODE == "dr0":
        # DoubleRow lhsT [128, 2*128]: both halves Hd/2; the rhs h axis
        # is a zero-stride broadcast, so out = (Hd/2 + Hd/2) @ D.
        return np.concatenate([Hd / 2, Hd / 2], axis=1).astype(IN_NP)
    return Hd.astype(IN_NP)  # [128, 128]


def _build_program() -> bacc.Bacc:
    nc = bacc.Bacc("TRN2", target_bir_lowering=False, debug=False,
                   num_devices=N_CORES)
    x = nc.dram_tensor("x", [TILES, 128, COLS], IN_DT,
                       kind="ExternalInput").ap()
    wshape = [128, 256] if MM_MODE == "dr0" else [128, 128]
    w = nc.dram_tensor("w", wshape, IN_DT, kind="ExternalInput").ap()
    out = nc.dram_tensor("out", [128, 2], F32, kind="ExternalOutput").ap()

    nquads = TILES * QPT                     # 32
    # Stable interleave of the DVE/Act split across the quad stream.
    dve_q = {i for i in range(nquads)
             if (i + 1) * DVE_QUADS // nquads > i * DVE_QUADS // nquads}
    n_dve = len(dve_q)
    n_act = nquads - n_dve

    with tile.TileContext(nc) as tc, ExitStack() as ctx:
        wpool = ctx.enter_context(tc.tile_pool(name="w", bufs=1))
        xpool = ctx.enter_context(tc.tile_pool(name="x", bufs=4))
        psum = ctx.enter_context(tc.tile_pool(name="psum", bufs=4,
                                              space="PSUM"))
        accpool = ctx.enter_context(tc.tile_pool(name="acc", bufs=1))
        scratch = ctx.enter_context(tc.tile_pool(name="scr", bufs=2))

        wt = wpool.tile(wshape, IN_DT)
        nc.sync.dma_start(wt[:], w[:])
        if MM_MODE == "dr0":
            w3 = wt[:].rearrange("p (h m) -> p h m", h=2)

        # Separate accumulators per reduce engine so VectorE and ScalarE
        # never touch the same tile (no cross-engine serialization).
        accv = accpool.tile([128, n_dve], F32, tag="accv")
        acca = accpool.tile([128, n_act], F32, tag="acca")

        iv = ia = 0
        for t in range(TILES):
            xt = xpool.tile([128, COLS], IN_DT)
            # First tile streams in chunks for faster pipeline fill.
            n_chunks = 4 if t == 0 else 1
            step = COLS // n_chunks
            for c0 in range(0, COLS, step):
                nc.sync.dma_start(xt[:, c0:c0 + step], x[t, :, c0:c0 + step])
            for qq in range(QPT):
                pt = psum.tile([128, QUAD], F32)
                for s in range(QUAD // MM_N):
                    lo = qq * QUAD + s * MM_N
                    xs = xt[:, lo:lo + MM_N]
                    po = pt[:, s * MM_N:(s + 1) * MM_N]
                    if MM_MODE == "dr0":
                        x3 = xs.unsqueeze(1).broadcast_to([128, 2, MM_N])
                        nc.tensor.matmul(
                            po, w3, x3, start=True, stop=True,
                            perf_mode=mybir.MatmulPerfMode.DoubleRow)
                    else:
                        nc.tensor.matmul(po, wt[:], xs, start=True,
                                         stop=True)
                k = t * QPT + qq
                if k in dve_q:
                    nc.vector.tensor_reduce(
                        accv[:, iv:iv + 1], pt[:],
                        axis=mybir.AxisListType.X, op=mybir.AluOpType.add,
                        apply_absolute_value=True)
                    iv += 1
                else:
                    st = scratch.tile([128, QUAD], mybir.dt.bfloat16)
                    nc.scalar.activation(
                        st[:], pt[:], mybir.ActivationFunctionType.Abs,
                        accum_out=acca[:, ia:ia + 1])
                    ia += 1

        res = accpool.tile([128, 2], F32, tag="res")
        nc.vector.tensor_reduce(res[:, 0:1], accv[:],
                                axis=mybir.AxisListType.X,
                                op=mybir.AluOpType.add)
        nc.vector.tensor_reduce(res[:, 1:2], acca[:],
                                axis=mybir.AxisListType.X,
                                op=mybir.AluOpType.add)
        nc.sync.dma_start(out[:], res[:])

    nc.compile()
    return nc


def _repack(shard: np.ndarray) -> np.ndarray:
    """[BLOCKS_PER_CORE, 64, 64] f32 -> [TILES, 128, COLS] fp8 with
    partition axis (m, j) and free axis (g, k)."""
    v = shard.reshape(TILES, 2, G, N, N)          # t, m, g, j, k
    v = v.transpose(0, 1, 3, 2, 4)                # t, m, j, g, k
    return np.ascontiguousarray(v).reshape(TILES, 128, COLS).astype(IN_NP)


_NC = None


def _get_program() -> bacc.Bacc:
    global _NC
    if _NC is None:
        _NC = _build_program()
    return _NC


def _run(original: np.ndarray, pred: np.ndarray, **spmd_kwargs):
    diff = np.asarray(original, dtype=np.float32).reshape(
        BLOCKS_TOTAL, N, N) - np.asarray(pred, dtype=np.float32).reshape(
        BLOCKS_TOTAL, N, N)
    wnp = _weights()
    in_maps = []
    for i in range(N_CORES):
        sl = slice(i * BLOCKS_PER_CORE, (i + 1) * BLOCKS_PER_CORE)
        in_maps.append({"x": _repack(diff[sl]), "w": wnp})
    nc = _get_program()
    r = run_bass_kernel_spmd(nc, in_maps, list(range(N_CORES)),
                             **spmd_kwargs)
    total = 0.0
    for i in range(N_CORES):
        total += r.results[i]["out"].astype(np.float64).sum()
    return np.float32(total), r


def kernel(original: np.ndarray, pred: np.ndarray) -> np.ndarray:
    val, _ = _run(original, pred)
    return np.array(val, dtype=np.float32)
